# revision 12
# baseline (speedup 1.0000x reference)
"""
DistancePredictor Trainium2 kernel.

Math:
  xi = x @ Wi + bi            [B, L, H]
  xj = x @ Wj + bj            [B, L, H]
  out = relu(xi[:,:,None,:] * xj[:,None,:,:]) @ Wo + bo    [B, L, L, NB]

Key identity (exact in fp arithmetic, terms have disjoint support):
  relu(a*b) = relu(a)relu(b) + relu(-a)relu(-b)
            = max(a,0)max(b,0) + min(a,0)min(b,0)
so with P=max(.,0), M=min(.,0):
  out[i,j,n] = sum_h (Pi[i,h]*Pj[j,h] + Mi[i,h]*Mj[j,h]) * Wo[h,n] + bo[n]
The min-branch signs cancel, so no negation is needed anywhere.  Folding
Wo[:,n] into the (small) i-side makes the whole pair/relu/contract
pipeline pure TensorE matmuls — no [B,L,L,H] intermediate ever exists.

Sharding: 8 cores; core c handles batch b=c//4 and i-rows
[96*(c%4), 96*(c%4)+96).  Weights replicated.

Layout tricks:
 - x[b] is transposed AND rolled by -i0 on the host, so the core's own
   96 i-rows are columns 0:96 of its xbt — no separate sliced input.
   The j axis is therefore rolled per-core; undone during host unshard.
 - Wi|Wj are packed into one [D, 2H] tensor so each contraction chunk
   is a single DMA per ring (sync ring carries x, scalar ring carries W).
 - xiT is computed directly in [h, i] layout (lhsT = Wi chunk,
   rhs = x-rows chunk) — no on-chip transpose needed.
 - Output is produced in [NB, 96, L] layout (contiguous DMA per n-pair),
   transposed to [96, L, NB] on the host during unshard.
"""

import numpy as np

import concourse.bass as bass
import concourse.mybir as mybir
import concourse.tile as tile
from concourse import bacc, bass_utils

# Problem constants (hardcoded per contract).
B, L, D, H, NB = 2, 384, 1280, 256, 10
P = 128
KT = D // P     # 10 contraction chunks of 128
KC = 2          # k-chunks per DMA
HT = H // P     # 2 h-chunks of 128
NCORES = 8
IB = (B * L) // NCORES   # 96 i-rows per core

F32 = mybir.dt.float32
F32R = mybir.dt.float32r
ALU = mybir.AluOpType

_last_result = None  # BassKernelResults of the most recent run (for test harness)


def build_nc():
    nc = bacc.Bacc("TRN2")

    xbt = nc.dram_tensor("xbt", [D, L], F32R, kind="ExternalInput")    # roll(x[b].T, -i0)
    wij = nc.dram_tensor("wij", [D, 2 * H], F32R, kind="ExternalInput")  # [Wi | Wj]
    wo = nc.dram_tensor("wo", [P, HT, NB], F32, kind="ExternalInput")    # Wo[t*128+p, n]
    biases = nc.dram_tensor("biases", [P, HT, 2], F32, kind="ExternalInput")  # bi, bj
    bo_rep = nc.dram_tensor("bo_rep", [P, NB], F32, kind="ExternalInput")     # bo replicated
    out = nc.dram_tensor("out", [NB, IB, L], F32, kind="ExternalOutput")

    xbt_r = xbt[:].rearrange("(c k p) j -> p c k j", p=P, k=KC)   # [128, 5, 2, 384]
    wij_r = wij[:].rearrange("(c k p) h -> p c k h", p=P, k=KC)   # [128, 5, 2, 512]

    with tile.TileContext(nc) as tc:
        with (
            tc.tile_pool(name="persist", bufs=1) as pp,
            tc.tile_pool(name="psA", bufs=2, space="PSUM") as psA_pool,
            tc.tile_pool(name="psB", bufs=2, space="PSUM") as psB_pool,
            tc.tile_pool(name="psO", bufs=3, space="PSUM") as psO_pool,
            tc.tile_pool(name="stage", bufs=3) as stage_pool,
            tc.tile_pool(name="apm", bufs=2) as apm_pool,
        ):
            # ---- persistent SBUF tiles ----
            tl = lambda shape, name, dt=F32: pp.tile(shape, dt, name=name, tag=name)
            xbt_sb = tl([P, KT, L], "xbt_sb", F32R)
            wij_sb = tl([P, KT, 2 * H], "wij_sb", F32R)
            wo_sb = tl([P, HT, NB], "wo_sb")
            bias_sb = tl([P, HT, 2], "bias_sb")
            bo_sb = tl([P, NB], "bo_sb")

            bp_sb = tl([P, HT, L], "bp_sb", F32R)     # max(xj+bj, 0)   [h, j]
            bm_sb = tl([P, HT, L], "bm_sb", F32R)     # min(xj+bj, 0)
            atp_sb = tl([P, HT, NB, IB], "atp_sb", F32R)  # max(xi+bi,0) * Wo  [h, n, i]
            atm_sb = tl([P, HT, NB, IB], "atm_sb", F32R)  # min(xi+bi,0) * Wo

            # ---- small constant DMAs (SWDGE; keeps both HW rings free) ----
            nc.gpsimd.dma_start(wo_sb[:], wo[:])
            nc.gpsimd.dma_start(bias_sb[:], biases[:])
            nc.gpsimd.dma_start(bo_sb[:], bo_rep[:])

            psA0 = psA_pool.tile([P, IB], F32, name="psA", tag="psA")
            psA1 = psA_pool.tile([P, IB], F32, name="psA", tag="psA")
            psA = [psA0, psA1]
            psB0 = psB_pool.tile([P, L], F32, name="psB", tag="psB")
            psB1 = psB_pool.tile([P, L], F32, name="psB", tag="psB")
            psB = [psB0, psB1]

            # ---- input DMAs: two HWDGE rings in parallel, 2k-sized chunks ----
            xbt_c = xbt_sb[:].rearrange("p (c k) j -> p c k j", k=KC)
            wij_c = wij_sb[:].rearrange("p (c k) h -> p c k h", k=KC)
            for c in range(KT // KC):
                nc.sync.dma_start(xbt_c[:, c], xbt_r[:, c])
                nc.scalar.dma_start(wij_c[:, c], wij_r[:, c])

            # ---- first layer: accumulate over k in PSUM ----
            for k in range(KT):
                st, sp = (k == 0), (k == KT - 1)
                for t in range(HT):
                    # xiT[t][h, i] += wi_k[:, t].T @ x_rows_k   (N=96)
                    nc.tensor.matmul(psA[t][:],
                                     wij_sb[:, k, t * P:(t + 1) * P],
                                     xbt_sb[:, k, :IB],
                                     start=st, stop=sp)
                    # xjT[t][h, j] += wj_k[:, t].T @ xbt_k      (N=384)
                    nc.tensor.matmul(psB[t][:],
                                     wij_sb[:, k, H + t * P:H + (t + 1) * P],
                                     xbt_sb[:, k, :],
                                     start=st, stop=sp)

            # ---- A side: a± = max/min(xiT + bi, 0); at± = a± * Wo[:,n] ----
            ap_ts, am_ts = [], []
            for t in range(HT):
                ap_t = apm_pool.tile([P, IB], F32, name="ap_t", tag=f"ap_t{t}")
                am_t = apm_pool.tile([P, IB], F32, name="am_t", tag=f"am_t{t}")
                nc.vector.tensor_scalar(ap_t[:], psA[t][:],
                                        bias_sb[:, t, 0:1], 0.0,
                                        ALU.add, ALU.max)
                nc.vector.tensor_scalar(am_t[:], psA[t][:],
                                        bias_sb[:, t, 0:1], 0.0,
                                        ALU.add, ALU.min)
                ap_ts.append(ap_t)
                am_ts.append(am_t)

            # split by n-halves so the first main matmuls start early
            NH = NB // 2
            for half in range(2):
                ns = slice(half * NH, (half + 1) * NH)
                for t in range(HT):
                    nc.vector.tensor_tensor(
                        atp_sb[:, t, ns],
                        ap_ts[t][:, None, :].to_broadcast((P, NH, IB)),
                        wo_sb[:, t, ns, None].to_broadcast((P, NH, IB)),
                        ALU.mult)
                    nc.vector.tensor_tensor(
                        atm_sb[:, t, ns],
                        am_ts[t][:, None, :].to_broadcast((P, NH, IB)),
                        wo_sb[:, t, ns, None].to_broadcast((P, NH, IB)),
                        ALU.mult)
                if half == 0:
                    # B side for t=0/1 needed by the first main matmuls
                    for t in range(HT):
                        nc.vector.tensor_scalar(bp_sb[:, t, :], psB[t][:],
                                                bias_sb[:, t, 1:2], 0.0,
                                                ALU.add, ALU.max)
                        nc.vector.tensor_scalar(bm_sb[:, t, :], psB[t][:],
                                                bias_sb[:, t, 1:2], 0.0,
                                                ALU.add, ALU.min)

            # ---- main contraction: per n, 4 accumulating matmuls ----
            out_pair = out[:].rearrange("(np two) i j -> np i two j", two=2)
            for np_ in range(NB // 2):
                ostage = stage_pool.tile([IB, 2, L], F32, name="ostage", tag="ostage")
                for par in range(2):
                    n = np_ * 2 + par
                    psO = psO_pool.tile([IB, L], F32, name="psO", tag="psO")
                    nc.tensor.matmul(psO[:], atp_sb[:, 0, n, :], bp_sb[:, 0, :],
                                     start=True, stop=False)
                    nc.tensor.matmul(psO[:], atp_sb[:, 1, n, :], bp_sb[:, 1, :],
                                     start=False, stop=False)
                    nc.tensor.matmul(psO[:], atm_sb[:, 0, n, :], bm_sb[:, 0, :],
                                     start=False, stop=False)
                    nc.tensor.matmul(psO[:], atm_sb[:, 1, n, :], bm_sb[:, 1, :],
                                     start=False, stop=True)
                    nc.vector.tensor_scalar_add(ostage[:, par, :], psO[:],
                                                bo_sb[:IB, n:n + 1])
                eng = nc.sync if np_ % 2 == 0 else nc.scalar
                eng.dma_start(out_pair[np_], ostage[:])

    return nc


def _prep_inputs(x, Wi, bi, Wj, bj, Wo, bo):
    """Build the 8 per-core input maps."""
    f = lambda a: np.ascontiguousarray(np.asarray(a, dtype=np.float32))
    x, Wi, bi, Wj, bj, Wo, bo = map(f, (x, Wi, bi, Wj, bj, Wo, bo))

    wij = np.ascontiguousarray(np.hstack([Wi, Wj]))                        # [D, 512]
    wo_r = np.ascontiguousarray(Wo.reshape(HT, P, NB).transpose(1, 0, 2))  # [128, 2, 10]
    br = lambda v: v.reshape(HT, P).T                                      # [128, 2]
    biases = np.ascontiguousarray(np.stack([br(bi), br(bj)], axis=2))      # [128, 2, 2]
    bo_rep = np.ascontiguousarray(np.tile(bo[None, :], (P, 1)))            # [128, 10]

    xT = [np.ascontiguousarray(x[b].T) for b in range(B)]                  # [1280, 384]
    in_maps = []
    for c in range(NCORES):
        b, i0 = c // (NCORES // B), (c % (NCORES // B)) * IB
        in_maps.append({
            "xbt": np.ascontiguousarray(np.roll(xT[b], -i0, axis=1)),
            "wij": wij, "wo": wo_r, "biases": biases, "bo_rep": bo_rep,
        })
    return in_maps


def _run(inputs, trace=False):
    global _last_result
    nc = build_nc()
    if not nc.is_finalized():
        nc.finalize()
    in_maps = _prep_inputs(**inputs)
    res = bass_utils.run_bass_kernel_spmd(
        nc, in_maps, core_ids=list(range(NCORES)), trace=trace)
    _last_result = res
    full = np.empty((B, L, L, NB), dtype=np.float32)
    for c in range(NCORES):
        b, i0 = c // (NCORES // B), (c % (NCORES // B)) * IB
        o = res.results[c]["out"]          # [NB, IB, L], j rolled by -i0
        full[b, i0:i0 + IB] = np.roll(o, i0, axis=2).transpose(1, 2, 0)
    return full


def kernel(**inputs):
    return _run(inputs, trace=False)


# revision 13
# speedup vs baseline: 1.3591x; 1.3591x over previous
"""
DistancePredictor Trainium2 kernel.

Math:
  xi = x @ Wi + bi            [B, L, H]
  xj = x @ Wj + bj            [B, L, H]
  out = relu(xi[:,:,None,:] * xj[:,None,:,:]) @ Wo + bo    [B, L, L, NB]

Key identity (exact in fp arithmetic, terms have disjoint support):
  relu(a*b) = relu(a)relu(b) + relu(-a)relu(-b)
so
  out[i,j,n] = sum_h (A+[i,h]B+[j,h] + A-[i,h]B-[j,h]) * Wo[h,n] + bo[n]
with A± = relu(±xi), B± = relu(±xj).  This makes the whole
pair/relu/contract pipeline pure TensorE matmuls — no [B,L,L,H]
intermediate ever exists.  Signs are arranged as:
  A+ = max(xi+bi, 0) (DVE),  A- = min(xi+bi, 0)      (= -relu(-(xi+bi)))
  B+ = relu(xj+bj)   (ACT),  B-'= relu(-(xj+bj))     (= -min)
  at+ = A+ * Wo,  at- = A- * (-Wo)   so  at-·B-' = relu(-xi)relu(-xj)·Wo.

Sharding: 8 cores; core c handles batch b=c//4 and i-rows
[96*(c%4), 96*(c%4)+96).  Weights replicated.

Layout tricks:
 - x[b] is transposed AND rolled by -i0 on the host, so the core's own
   96 i-rows are columns 0:96 of its xbt — no separate sliced input.
   The j axis is therefore rolled per-core; undone during host unshard.
 - First layer runs in bf16 (host-cast): halves input DMA bytes; the
   second layer runs fp32r from on-chip fp32 PSUM results.
 - Wi|Wj are packed into one [D, 2H] tensor so each contraction chunk
   is a single DMA per ring (sync ring carries x, scalar ring carries W).
 - xiT is computed directly in [h, i] layout — no on-chip transpose.
 - Output is produced in [NB, 96, L] layout (contiguous DMA per n-pair),
   transposed to [96, L, NB] on the host during unshard.
 - A burst of dummy matmuls on already-landed data right after the first
   chunk keeps TensorE continuously busy so the PE HAM clock un-throttles
   (1.2 -> 2.4 GHz) before the real work peaks.
"""

import numpy as np
import ml_dtypes

import concourse.bass as bass
import concourse.mybir as mybir
import concourse.tile as tile
from concourse import bacc, bass_utils

# Problem constants (hardcoded per contract).
B, L, D, H, NB = 2, 384, 1280, 256, 10
P = 128
KT = D // P     # 10 contraction chunks of 128
KC = 2          # k-chunks per DMA
HT = H // P     # 2 h-chunks of 128
NCORES = 8
IB = (B * L) // NCORES   # 96 i-rows per core
N_WARM = 10     # HAM warm-up dummy matmuls

F32 = mybir.dt.float32
F32R = mybir.dt.float32r
BF16 = mybir.dt.bfloat16
ALU = mybir.AluOpType
RELU = mybir.ActivationFunctionType.Relu

_last_result = None  # BassKernelResults of the most recent run (for test harness)


def build_nc():
    nc = bacc.Bacc("TRN2")

    xbt = nc.dram_tensor("xbt", [D, L], BF16, kind="ExternalInput")    # roll(x[b].T, -i0)
    wij = nc.dram_tensor("wij", [D, 2 * H], BF16, kind="ExternalInput")  # [Wi | Wj]
    wo2 = nc.dram_tensor("wo2", [P, 2, HT, NB], F32, kind="ExternalInput")  # [Wo, -Wo]
    biases = nc.dram_tensor("biases", [P, HT, 3], F32, kind="ExternalInput")  # bi, bj, -bj
    bo_rep = nc.dram_tensor("bo_rep", [P, NB], F32, kind="ExternalInput")     # bo replicated
    out = nc.dram_tensor("out", [NB, IB, L], F32, kind="ExternalOutput")

    xbt_r = xbt[:].rearrange("(c k p) j -> p c k j", p=P, k=KC)   # [128, 5, 2, 384]
    wij_r = wij[:].rearrange("(c k p) h -> p c k h", p=P, k=KC)   # [128, 5, 2, 512]

    with tile.TileContext(nc) as tc:
        with (
            tc.tile_pool(name="persist", bufs=1) as pp,
            tc.tile_pool(name="psA", bufs=2, space="PSUM") as psA_pool,
            tc.tile_pool(name="psB", bufs=2, space="PSUM") as psB_pool,
            tc.tile_pool(name="psO", bufs=3, space="PSUM") as psO_pool,
            tc.tile_pool(name="psW", bufs=1, space="PSUM") as psW_pool,
            tc.tile_pool(name="stage", bufs=3) as stage_pool,
            tc.tile_pool(name="apm", bufs=2) as apm_pool,
        ):
            # ---- persistent SBUF tiles ----
            tl = lambda shape, name, dt=F32: pp.tile(shape, dt, name=name, tag=name)
            xbt_sb = tl([P, KT, L], "xbt_sb", BF16)
            wij_sb = tl([P, KT, 2 * H], "wij_sb", BF16)
            wo_sb = tl([P, 2, HT, NB], "wo_sb")
            bias_sb = tl([P, HT, 3], "bias_sb")
            bo_sb = tl([P, NB], "bo_sb")

            bp_sb = tl([P, HT, L], "bp_sb", F32R)     # relu(xj+bj)      [h, j]
            bm_sb = tl([P, HT, L], "bm_sb", F32R)     # relu(-(xj+bj))
            atp_sb = tl([P, HT, NB, IB], "atp_sb", F32R)  # max(xi+bi,0) *  Wo  [h, n, i]
            atm_sb = tl([P, HT, NB, IB], "atm_sb", F32R)  # min(xi+bi,0) * -Wo

            # ---- small constant DMAs (SWDGE; keeps both HW rings free) ----
            nc.gpsimd.dma_start(wo_sb[:], wo2[:])
            nc.gpsimd.dma_start(bias_sb[:], biases[:])
            nc.gpsimd.dma_start(bo_sb[:], bo_rep[:])

            psA0 = psA_pool.tile([P, IB], F32, name="psA", tag="psA")
            psA1 = psA_pool.tile([P, IB], F32, name="psA", tag="psA")
            psA = [psA0, psA1]
            psB0 = psB_pool.tile([P, L], F32, name="psB", tag="psB")
            psB1 = psB_pool.tile([P, L], F32, name="psB", tag="psB")
            psB = [psB0, psB1]

            # ---- input DMAs: two HWDGE rings in parallel, 2k-sized chunks ----
            xbt_c = xbt_sb[:].rearrange("p (c k) j -> p c k j", k=KC)
            wij_c = wij_sb[:].rearrange("p (c k) h -> p c k h", k=KC)
            for c in range(KT // KC):
                nc.sync.dma_start(xbt_c[:, c], xbt_r[:, c])
                nc.scalar.dma_start(wij_c[:, c], wij_r[:, c])

            # ---- first layer: accumulate over k in PSUM ----
            for k in range(KT):
                st, sp = (k == 0), (k == KT - 1)
                for t in range(HT):
                    # xiT[t][h, i] += wi_k[:, t].T @ x_rows_k   (N=96)
                    nc.tensor.matmul(psA[t][:],
                                     wij_sb[:, k, t * P:(t + 1) * P],
                                     xbt_sb[:, k, :IB],
                                     start=st, stop=sp)
                    # xjT[t][h, j] += wj_k[:, t].T @ xbt_k      (N=384)
                    nc.tensor.matmul(psB[t][:],
                                     wij_sb[:, k, H + t * P:H + (t + 1) * P],
                                     xbt_sb[:, k, :],
                                     start=st, stop=sp)
                if k == 1:
                    # HAM warm-up: dummy matmuls on chunk-0 data keep the PE
                    # continuously busy through the DMA-bound phase so the
                    # clock gate opens (2.4 GHz) before the compute peak.
                    psW = psW_pool.tile([P, L], F32, name="psW")
                    for w in range(N_WARM):
                        nc.tensor.matmul(psW[:], wij_sb[:, 0, 0:P],
                                         xbt_sb[:, 0, :],
                                         start=True, stop=True,
                                         skip_group_check=True)

            # ---- A side on DVE: a+ = max(xiT+bi, 0), a- = min(xiT+bi, 0) ----
            ap_ts, am_ts = [], []
            for t in range(HT):
                ap_t = apm_pool.tile([P, IB], F32, name="ap_t", tag=f"ap_t{t}")
                am_t = apm_pool.tile([P, IB], F32, name="am_t", tag=f"am_t{t}")
                nc.vector.tensor_scalar(ap_t[:], psA[t][:],
                                        bias_sb[:, t, 0:1], 0.0,
                                        ALU.add, ALU.max)
                nc.vector.tensor_scalar(am_t[:], psA[t][:],
                                        bias_sb[:, t, 0:1], 0.0,
                                        ALU.add, ALU.min)
                ap_ts.append(ap_t)
                am_ts.append(am_t)

            # ---- B side on ACT: bp = relu(xj+bj), bm = relu(-(xj+bj)) ----
            for t in range(HT):
                nc.scalar.activation(bp_sb[:, t, :], psB[t][:], RELU,
                                     bias=bias_sb[:, t, 1:2], scale=1.0)
                nc.scalar.activation(bm_sb[:, t, :], psB[t][:], RELU,
                                     bias=bias_sb[:, t, 2:3], scale=-1.0)

            # at±[h, n, i] = a±[h, i] * (±Wo)[h, n], split by n-halves so the
            # first main matmuls can start early.
            NH = NB // 2
            for half in range(2):
                ns = slice(half * NH, (half + 1) * NH)
                for t in range(HT):
                    nc.vector.tensor_tensor(
                        atp_sb[:, t, ns],
                        ap_ts[t][:, None, :].to_broadcast((P, NH, IB)),
                        wo_sb[:, 0, t, ns, None].to_broadcast((P, NH, IB)),
                        ALU.mult)
                    nc.vector.tensor_tensor(
                        atm_sb[:, t, ns],
                        am_ts[t][:, None, :].to_broadcast((P, NH, IB)),
                        wo_sb[:, 1, t, ns, None].to_broadcast((P, NH, IB)),
                        ALU.mult)

            # ---- main contraction: per n, 4 accumulating matmuls ----
            out_pair = out[:].rearrange("(np two) i j -> np i two j", two=2)
            for np_ in range(NB // 2):
                ostage = stage_pool.tile([IB, 2, L], F32, name="ostage", tag="ostage")
                for par in range(2):
                    n = np_ * 2 + par
                    psO = psO_pool.tile([IB, L], F32, name="psO", tag="psO")
                    nc.tensor.matmul(psO[:], atp_sb[:, 0, n, :], bp_sb[:, 0, :],
                                     start=True, stop=False)
                    nc.tensor.matmul(psO[:], atp_sb[:, 1, n, :], bp_sb[:, 1, :],
                                     start=False, stop=False)
                    nc.tensor.matmul(psO[:], atm_sb[:, 0, n, :], bm_sb[:, 0, :],
                                     start=False, stop=False)
                    nc.tensor.matmul(psO[:], atm_sb[:, 1, n, :], bm_sb[:, 1, :],
                                     start=False, stop=True)
                    # + bo[n]: alternate engines so neither becomes critical
                    if par == 0:
                        nc.vector.tensor_scalar_add(ostage[:, par, :], psO[:],
                                                    bo_sb[:IB, n:n + 1])
                    else:
                        nc.scalar.activation(
                            ostage[:, par, :], psO[:],
                            mybir.ActivationFunctionType.Identity,
                            bias=bo_sb[:IB, n:n + 1], scale=1.0)
                eng = nc.sync if np_ % 2 == 0 else nc.scalar
                eng.dma_start(out_pair[np_], ostage[:])

    return nc


def _prep_inputs(x, Wi, bi, Wj, bj, Wo, bo):
    """Build the 8 per-core input maps."""
    f = lambda a: np.ascontiguousarray(np.asarray(a, dtype=np.float32))
    x, Wi, bi, Wj, bj, Wo, bo = map(f, (x, Wi, bi, Wj, bj, Wo, bo))

    wij = np.ascontiguousarray(np.hstack([Wi, Wj]).astype(ml_dtypes.bfloat16))
    wo_r = Wo.reshape(HT, P, NB).transpose(1, 0, 2)                        # [128, 2, 10]
    wo2 = np.ascontiguousarray(np.stack([wo_r, -wo_r], axis=1))            # [128, 2, 2, 10]
    br = lambda v: v.reshape(HT, P).T                                      # [128, 2]
    biases = np.ascontiguousarray(np.stack([br(bi), br(bj), -br(bj)], axis=2))
    bo_rep = np.ascontiguousarray(np.tile(bo[None, :], (P, 1)))            # [128, 10]

    xT = [x[b].T for b in range(B)]                                        # [1280, 384]
    in_maps = []
    for c in range(NCORES):
        b, i0 = c // (NCORES // B), (c % (NCORES // B)) * IB
        in_maps.append({
            "xbt": np.ascontiguousarray(
                np.roll(xT[b], -i0, axis=1).astype(ml_dtypes.bfloat16)),
            "wij": wij, "wo2": wo2, "biases": biases, "bo_rep": bo_rep,
        })
    return in_maps


def _run(inputs, trace=False):
    global _last_result
    nc = build_nc()
    if not nc.is_finalized():
        nc.finalize()
    in_maps = _prep_inputs(**inputs)
    res = bass_utils.run_bass_kernel_spmd(
        nc, in_maps, core_ids=list(range(NCORES)), trace=trace)
    _last_result = res
    full = np.empty((B, L, L, NB), dtype=np.float32)
    for c in range(NCORES):
        b, i0 = c // (NCORES // B), (c % (NCORES // B)) * IB
        o = res.results[c]["out"]          # [NB, IB, L], j rolled by -i0
        full[b, i0:i0 + IB] = np.roll(o, i0, axis=2).transpose(1, 2, 0)
    return full


def kernel(**inputs):
    return _run(inputs, trace=False)
